# revision 10
# baseline (speedup 1.0000x reference)
"""MoE (DeepSeek-style, no gate) SwiGLU kernel for 8 Trainium2 NeuronCores.

Strategy (expert parallelism, per the sharding hint):
  - 16 routed experts sharded 2-per-core across 8 cores.
  - Token dispatch done host-side: for each expert, gather the tokens routed
    to it (topk membership), pad to a common capacity C, and ship the
    pre-transposed activation columns xT[:, tokens] to the owning core.
  - Shared expert tensor-parallel over its inter dim (2816/8 = 352 cols per
    core, zero-padded to 384), computed on all 2048 tokens in 2 chunks.
  - Each core runs the same Bass program (SPMD) on its own shard; the host
    scatter-adds routed outputs and sums shared-expert partials.

Compute layout per phase (one phase = one SwiGLU MLP on a token set):
  phase 1:  h1T = W1p.T @ xT, h3T = W3p.T @ xT   (I on partitions, tokens free)
            h' = silu(h1T) * h3T                  -> bf16 in SBUF
  phase 2:  y[tok_tile] = h'.T @ W2p, scaled per-token by comb weight on
            PSUM eviction (tensor_scalar with a [128,1] per-partition scalar).

All weights/activations are cast to bf16 on host (halves HBM traffic; PE
runs bf16 at 1 cycle/row). PSUM accumulation is fp32; outputs are fp32.
Host pre-tiles every tensor so that every DMA is fully contiguous.
"""

import numpy as np
import ml_dtypes

import concourse.bass as bass
import concourse.bacc as bacc
import concourse.mybir as mybir
import concourse.tile as tile
from concourse.bass_utils import run_bass_kernel_spmd

BF16 = ml_dtypes.bfloat16
F32 = np.float32
P = 128
NSZ = 512  # PSUM bank free size (fp32)

FULL_CFG = dict(
    ncores=8,
    T=2048,
    D=2048,
    E=16,
    I_E=1408,
    ish_c=352,       # shared inter per core (2816 / 8)
    ish_pad=384,     # zero-padded to 3 partition tiles
    t_chunk=1024,    # shared-expert token chunk
    d_out=2048,
)


def _derived(cfg):
    return dict(
        epc=cfg["E"] // cfg["ncores"],
        kd=cfg["D"] // P,
        it_r=cfg["I_E"] // P,
        it_s=cfg["ish_pad"] // P,
        nch=cfg["T"] // cfg["t_chunk"],
    )


def _emit_phase(nc, pools, xt_dram, w1_dram, w3_dram, w2_dram, cg_dram,
                out_rows, n_itiles, cp, cfg, ph):
    """One SwiGLU MLP phase over `cp` token columns with `n_itiles` I-tiles.

    out_rows[mt] is the DRAM destination for token tile mt ([128, d_out]).
    cg_dram is None for the shared expert (no per-token combine weight).
    """
    kd = _derived(cfg)["kd"]
    d_out = cfg["d_out"]
    dt = mybir.dt.bfloat16
    f32 = mybir.dt.float32

    xtp, wp, hpp, w2p, sp, op, cgp, psA, psY = (
        pools["xt"], pools["w"], pools["hp"], pools["w2"], pools["s"],
        pools["o"], pools["cg"], pools["psA"], pools["psY"])

    xt = xtp.tile([P, kd, cp], dt, tag="xt", name=f"xt_{ph}")
    nc.sync.dma_start(out=xt[:], in_=xt_dram[:])

    cgt = None
    if cg_dram is not None:
        cgr = cgp.tile([P, cp // P], f32, tag="cgr", name=f"cgr_{ph}")
        nc.sync.dma_start(out=cgr[:], in_=cg_dram[:])
        # Bounce through DVE so the per-tile tensor_scalar evictions below
        # need only the PE wait (DVE has already observed the cg DMA here).
        cgt = cgp.tile([P, cp // P], f32, tag="cg", name=f"cg_{ph}")
        nc.vector.tensor_copy(cgt[:], cgr[:])

    # ---- phase 1: h' = silu(xW1) * (xW3), transposed layout [I, tokens] ----
    hp = []
    for m in range(n_itiles):
        w1t = wp.tile([P, kd, P], dt, tag="w1", name=f"w1_{ph}_{m}")
        nc.sync.dma_start(out=w1t[:], in_=w1_dram[m])
        w3t = wp.tile([P, kd, P], dt, tag="w3", name=f"w3_{ph}_{m}")
        nc.sync.dma_start(out=w3t[:], in_=w3_dram[m])
        hpm = hpp.tile([P, cp], dt, tag=f"hp_{m}", name=f"hp_{ph}_{m}")
        for n0 in range(0, cp, NSZ):
            nsz = min(NSZ, cp - n0)
            p1 = psA.tile([P, nsz], f32, tag="p1", name=f"p1_{ph}_{m}_{n0}")
            p3 = psA.tile([P, nsz], f32, tag="p3", name=f"p3_{ph}_{m}_{n0}")
            for kt in range(kd):
                nc.tensor.matmul(p1[:], w1t[:, kt, :],
                                 xt[:, kt, n0:n0 + nsz],
                                 start=(kt == 0), stop=(kt == kd - 1))
            for kt in range(kd):
                nc.tensor.matmul(p3[:], w3t[:, kt, :],
                                 xt[:, kt, n0:n0 + nsz],
                                 start=(kt == 0), stop=(kt == kd - 1))
            # silu(h1)*h3 = sigmoid(h1)*h3*h1 (CoreSim has no Silu LUT).
            # Both ACT ops read PSUM (wait on PE); both DVE ops then wait on
            # a single engine each — the DVE TensorTensor encoding only has
            # room for one sync-wait command.
            s = sp.tile([P, nsz], f32, tag="s", name=f"s_{ph}_{m}_{n0}")
            nc.scalar.activation(s[:], p1[:],
                                 mybir.ActivationFunctionType.Sigmoid)
            c3 = sp.tile([P, nsz], f32, tag="c3", name=f"c3_{ph}_{m}_{n0}")
            nc.scalar.copy(c3[:], p3[:])
            t = sp.tile([P, nsz], f32, tag="t", name=f"t_{ph}_{m}_{n0}")
            nc.vector.tensor_mul(t[:], s[:], c3[:])
            nc.vector.tensor_mul(hpm[:, n0:n0 + nsz], t[:], p1[:])
        hp.append(hpm)

    # ---- phase 2: out[tok] = comb * (h'.T @ W2) ----
    w2t = []
    for kt in range(n_itiles):
        w = w2p.tile([P, d_out], dt, tag=f"w2_{kt}", name=f"w2_{ph}_{kt}")
        nc.sync.dma_start(out=w[:], in_=w2_dram[kt])
        w2t.append(w)

    for mt in range(cp // P):
        osb = op.tile([P, d_out], f32, tag="osb", name=f"osb_{ph}_{mt}")
        for n0 in range(0, d_out, NSZ):
            nn = min(NSZ, d_out - n0)
            py = psY.tile([P, nn], f32, tag="py", name=f"py_{ph}_{mt}_{n0}")
            for kt in range(n_itiles):
                nc.tensor.matmul(py[:], hp[kt][:, mt * P:(mt + 1) * P],
                                 w2t[kt][:, n0:n0 + nn],
                                 start=(kt == 0), stop=(kt == n_itiles - 1))
            if cgt is not None:
                nc.vector.tensor_scalar_mul(osb[:, n0:n0 + nn], py[:],
                                            cgt[:, mt:mt + 1])
            else:
                nc.vector.tensor_copy(osb[:, n0:n0 + nn], py[:])
        nc.sync.dma_start(out=out_rows[mt], in_=osb[:])


def build_program(C, cfg):
    """Build the per-core Bass program. C = routed token capacity (mult of 128)."""
    nc = bacc.Bacc()
    dt = mybir.dt.bfloat16
    f32 = mybir.dt.float32
    dv = _derived(cfg)
    epc, kd, it_r, it_s, nch = dv["epc"], dv["kd"], dv["it_r"], dv["it_s"], dv["nch"]
    tch = cfg["t_chunk"]
    d_out = cfg["d_out"]

    ins = {}
    for j in range(epc):
        ins[f"xt{j}"] = nc.dram_tensor(f"xt{j}", [P, kd, C], dt, kind="ExternalInput")
        ins[f"w1_{j}"] = nc.dram_tensor(f"w1_{j}", [it_r, P, kd, P], dt, kind="ExternalInput")
        ins[f"w3_{j}"] = nc.dram_tensor(f"w3_{j}", [it_r, P, kd, P], dt, kind="ExternalInput")
        ins[f"w2_{j}"] = nc.dram_tensor(f"w2_{j}", [it_r, P, d_out], dt, kind="ExternalInput")
        ins[f"cg{j}"] = nc.dram_tensor(f"cg{j}", [P, C // P], f32, kind="ExternalInput")
    for ch in range(nch):
        ins[f"xts{ch}"] = nc.dram_tensor(f"xts{ch}", [P, kd, tch], dt, kind="ExternalInput")
    ins["ws1"] = nc.dram_tensor("ws1", [it_s, P, kd, P], dt, kind="ExternalInput")
    ins["ws3"] = nc.dram_tensor("ws3", [it_s, P, kd, P], dt, kind="ExternalInput")
    ins["ws2"] = nc.dram_tensor("ws2", [it_s, P, d_out], dt, kind="ExternalInput")

    outs = {}
    for j in range(epc):
        outs[f"y{j}"] = nc.dram_tensor(f"y{j}", [C // P, P, d_out], f32, kind="ExternalOutput")
    outs["z"] = nc.dram_tensor("z", [cfg["T"] // P, P, d_out], f32, kind="ExternalOutput")

    with tile.TileContext(nc) as tc:
        with (
            tc.tile_pool(name="xt", bufs=2) as xtp,
            tc.tile_pool(name="w", bufs=2) as wp,
            tc.tile_pool(name="hp", bufs=1) as hpp,
            tc.tile_pool(name="w2", bufs=1) as w2p,
            tc.tile_pool(name="s", bufs=3) as sp,
            tc.tile_pool(name="o", bufs=2) as op,
            tc.tile_pool(name="cg", bufs=2) as cgp,
            tc.tile_pool(name="psA", bufs=2, space="PSUM") as psA,
            tc.tile_pool(name="psY", bufs=4, space="PSUM") as psY,
        ):
            pools = dict(xt=xtp, w=wp, hp=hpp, w2=w2p, s=sp, o=op, cg=cgp,
                         psA=psA, psY=psY)
            for j in range(epc):
                _emit_phase(nc, pools, ins[f"xt{j}"], ins[f"w1_{j}"],
                            ins[f"w3_{j}"], ins[f"w2_{j}"], ins[f"cg{j}"],
                            [outs[f"y{j}"][mt] for mt in range(C // P)],
                            it_r, C, cfg, ph=f"e{j}")
            for ch in range(nch):
                mts = tch // P
                _emit_phase(nc, pools, ins[f"xts{ch}"], ins["ws1"],
                            ins["ws3"], ins["ws2"], None,
                            [outs["z"][ch * mts + mt] for mt in range(mts)],
                            it_s, tch, cfg, ph=f"s{ch}")
    nc.compile()
    return nc


def _panelize_w13(w, itiles):
    """(D, I) -> (itiles, 128, kd, 128) so each M-panel is one contiguous DMA."""
    dd, ii = w.shape
    return np.ascontiguousarray(
        w.reshape(dd // P, P, itiles, P).transpose(2, 1, 0, 3))


def prep(x, weights, indices, W1, W3, W2, Ws1, Ws3, Ws2, cfg):
    """Host-side dispatch: shard/gather/pad/cast/pre-tile all inputs."""
    T, D, E = cfg["T"], cfg["D"], cfg["E"]
    dv = _derived(cfg)
    epc, kd, it_r, it_s, nch = dv["epc"], dv["kd"], dv["it_r"], dv["it_s"], dv["nch"]
    tch = cfg["t_chunk"]
    ish_c, ish_pad = cfg["ish_c"], cfg["ish_pad"]

    xf = np.asarray(x, F32).reshape(T, D)
    wts = np.asarray(weights, F32)
    idx = np.asarray(indices).astype(np.int64)
    W1 = np.asarray(W1, F32)
    W3 = np.asarray(W3, F32)
    W2 = np.asarray(W2, F32)
    Ws1 = np.asarray(Ws1, F32)
    Ws3 = np.asarray(Ws3, F32)
    Ws2 = np.asarray(Ws2, F32)

    # Per-(token, expert) combine weight; duplicate expert ids accumulate.
    comb = np.zeros((T, E), F32)
    np.add.at(comb, (np.arange(T)[:, None], idx), wts)

    # Token dispatch (host-side all-to-all): gather token ids per expert.
    tok_lists = [np.nonzero((idx == e).any(axis=1))[0] for e in range(E)]
    counts = [len(t) for t in tok_lists]
    C = max(NSZ, -(-max(counts) // P) * P)

    xT16 = np.ascontiguousarray(xf.T).astype(BF16)  # (D, T)

    in_maps = []
    for c in range(cfg["ncores"]):
        m = {}
        for j in range(epc):
            e = epc * c + j
            toks = tok_lists[e]
            tpad = np.zeros(C, np.int64)
            tpad[:counts[e]] = toks
            xte = xT16[:, tpad].reshape(kd, P, C).transpose(1, 0, 2)
            m[f"xt{j}"] = np.ascontiguousarray(xte)
            m[f"w1_{j}"] = _panelize_w13(W1[e], it_r).astype(BF16)
            m[f"w3_{j}"] = _panelize_w13(W3[e], it_r).astype(BF16)
            m[f"w2_{j}"] = np.ascontiguousarray(W2[e].reshape(it_r, P, cfg["d_out"])).astype(BF16)
            cg = np.zeros(C, F32)
            cg[:counts[e]] = comb[toks, e]
            m[f"cg{j}"] = np.ascontiguousarray(cg.reshape(C // P, P).T)
        for ch in range(nch):
            xts = xT16[:, ch * tch:(ch + 1) * tch].reshape(kd, P, tch)
            m[f"xts{ch}"] = np.ascontiguousarray(xts.transpose(1, 0, 2))
        s0 = c * ish_c
        ws1 = np.zeros((D, ish_pad), F32)
        ws1[:, :ish_c] = Ws1[:, s0:s0 + ish_c]
        ws3 = np.zeros((D, ish_pad), F32)
        ws3[:, :ish_c] = Ws3[:, s0:s0 + ish_c]
        ws2 = np.zeros((ish_pad, cfg["d_out"]), F32)
        ws2[:ish_c] = Ws2[s0:s0 + ish_c]
        m["ws1"] = _panelize_w13(ws1, it_s).astype(BF16)
        m["ws3"] = _panelize_w13(ws3, it_s).astype(BF16)
        m["ws2"] = np.ascontiguousarray(ws2.reshape(it_s, P, cfg["d_out"])).astype(BF16)
        in_maps.append(m)

    meta = dict(tok_lists=tok_lists, counts=counts, C=C)
    return in_maps, meta


def combine(results, meta, cfg):
    """Host-side unshard: sum shared partials, scatter-add routed outputs."""
    T, D = cfg["T"], cfg["d_out"]
    epc = _derived(cfg)["epc"]
    C = meta["C"]
    out = np.zeros((T, D), F32)
    for c in range(cfg["ncores"]):
        r = results[c]
        out += r["z"].reshape(T, D)
        for j in range(epc):
            e = epc * c + j
            ye = r[f"y{j}"].reshape(C, D)[:meta["counts"][e]]
            out[meta["tok_lists"][e]] += ye
    return out


# Test-harness knobs (kernel() callers get no-trace defaults).
TRACE = False
TMPDIR = None
LAST_RESULT = None


def kernel(x, weights, indices, W1, W3, W2, Ws1, Ws3, Ws2):
    global LAST_RESULT
    cfg = FULL_CFG
    in_maps, meta = prep(x, weights, indices, W1, W3, W2,
                         Ws1, Ws3, Ws2, cfg)
    nc = build_program(meta["C"], cfg)
    res = run_bass_kernel_spmd(nc, in_maps, core_ids=list(range(cfg["ncores"])),
                               trace=TRACE, tmpdir=TMPDIR)
    LAST_RESULT = res
    out = combine(res.results, meta, cfg)
    return out.reshape(1, cfg["T"], cfg["D"]).astype(F32)


# revision 21
# speedup vs baseline: 1.0731x; 1.0731x over previous
"""MoE (DeepSeek-style, no gate) SwiGLU kernel for 8 Trainium2 NeuronCores.

Strategy (expert parallelism, per the sharding hint):
  - 16 routed experts sharded 2-per-core across 8 cores.
  - Token dispatch done host-side: for each expert, gather the tokens routed
    to it (topk membership), pad to a common capacity C, and ship the
    pre-transposed activation columns xT[:, tokens] to the owning core.
  - Shared expert tensor-parallel over its inter dim (2816/8 = 352 cols per
    core, zero-padded to 384), computed on all 2048 tokens in 2 chunks.
  - Each core runs the same Bass program (SPMD) on its own shard; the host
    scatter-adds routed outputs and sums shared-expert partials.

Compute layout per phase (one phase = one SwiGLU MLP on a token set):
  phase 1:  h1T = W1p.T @ xT, h3T = W3p.T @ xT   (I on partitions, tokens free)
            h' = silu(h1T) * h3T                  -> bf16 in SBUF
  phase 2:  y[tok_tile] = h'.T @ W2p, scaled per-token by comb weight on
            PSUM eviction (tensor_scalar with a [128,1] per-partition scalar).

All weights/activations are cast to bf16 on host (halves HBM traffic; PE
runs bf16 at 1 cycle/row). PSUM accumulation is fp32; outputs are fp32.
Host pre-tiles every tensor so that every DMA is fully contiguous.
"""

import numpy as np
import ml_dtypes

import concourse.bass as bass
import concourse.bacc as bacc
import concourse.mybir as mybir
import concourse.tile as tile
from concourse.bass_utils import run_bass_kernel_spmd

BF16 = ml_dtypes.bfloat16
F32 = np.float32
P = 128
NSZ = 512  # PSUM bank free size (fp32)

FULL_CFG = dict(
    ncores=8,
    T=2048,
    D=2048,
    E=16,
    I_E=1408,
    ish_c=352,       # shared inter per core (2816 / 8)
    ish_pad=384,     # zero-padded to 3 partition tiles
    t_chunk=1024,    # shared-expert token chunk
    d_out=2048,
)


def _derived(cfg):
    return dict(
        epc=cfg["E"] // cfg["ncores"],
        kd=cfg["D"] // P,
        it_r=cfg["I_E"] // P,
        it_s=cfg["ish_pad"] // P,
        nch=cfg["T"] // cfg["t_chunk"],
    )


def _emit_phase(nc, pools, xt_dram, w1_dram, w3_dram, w2_dram, cg_dram,
                out_rows, n_itiles, cp, cfg, ph):
    """One SwiGLU MLP phase over `cp` token columns with `n_itiles` I-tiles.

    out_rows[mt] is the DRAM destination for token tile mt ([128, d_out]).
    cg_dram is None for the shared expert (no per-token combine weight).
    """
    kd = _derived(cfg)["kd"]
    d_out = cfg["d_out"]
    dt = mybir.dt.bfloat16
    f32 = mybir.dt.float32

    xtp, wp, hpp, w2p, sp, op, cgp, psA, psY = (
        pools["xt"], pools["w"], pools["hp"], pools["w2"], pools["s"],
        pools["o"], pools["cg"], pools["psA"], pools["psY"])

    # DMA issue order = first-use order: xt k-tile 0, the m=0 weight panels,
    # then the remaining xt k-tiles, so the first matmul starts ASAP.
    xtk = [xtp.tile([P, cp], dt, tag="xt_0", name=f"xt_{ph}_0")]
    nc.sync.dma_start(out=xtk[0][:], in_=xt_dram[0])
    w1t0 = wp.tile([P, kd, P], dt, tag="w1", name=f"w1_{ph}_0")
    nc.sync.dma_start(out=w1t0[:], in_=w1_dram[0])
    w3t0 = wp.tile([P, kd, P], dt, tag="w3", name=f"w3_{ph}_0")
    nc.sync.dma_start(out=w3t0[:], in_=w3_dram[0])
    for kt in range(1, kd):
        x1 = xtp.tile([P, cp], dt, tag=f"xt_{kt}", name=f"xt_{ph}_{kt}")
        nc.sync.dma_start(out=x1[:], in_=xt_dram[kt])
        xtk.append(x1)

    cgt = None
    if cg_dram is not None:
        cg_mt = (cp + P - 1) // P
        cgr = cgp.tile([P, cg_mt], f32, tag="cgr", name=f"cgr_{ph}")
        nc.sync.dma_start(out=cgr[:], in_=cg_dram[:])
        # Bounce through DVE so the per-tile tensor_scalar evictions below
        # need only the PE wait (DVE has already observed the cg DMA here).
        cgt = cgp.tile([P, cg_mt], f32, tag="cg", name=f"cg_{ph}")
        nc.vector.tensor_copy(cgt[:], cgr[:])

    # ---- phase 1: h' = silu(xW1) * (xW3), transposed layout [I, tokens] ----
    hp = []
    for m in range(n_itiles):
        if m == 0:
            w1t, w3t = w1t0, w3t0
        else:
            w1t = wp.tile([P, kd, P], dt, tag="w1", name=f"w1_{ph}_{m}")
            nc.sync.dma_start(out=w1t[:], in_=w1_dram[m])
            w3t = wp.tile([P, kd, P], dt, tag="w3", name=f"w3_{ph}_{m}")
            nc.sync.dma_start(out=w3t[:], in_=w3_dram[m])
        hpm = hpp.tile([P, cp], dt, tag=f"hp_{m}", name=f"hp_{ph}_{m}")
        for n0 in range(0, cp, NSZ):
            nsz = min(NSZ, cp - n0)
            p1 = psA.tile([P, nsz], f32, tag="p1", name=f"p1_{ph}_{m}_{n0}")
            p3 = psA.tile([P, nsz], f32, tag="p3", name=f"p3_{ph}_{m}_{n0}")
            for kt in range(kd):
                nc.tensor.matmul(p1[:], w1t[:, kt, :],
                                 xtk[kt][:, n0:n0 + nsz],
                                 start=(kt == 0), stop=(kt == kd - 1))
            for kt in range(kd):
                nc.tensor.matmul(p3[:], w3t[:, kt, :],
                                 xtk[kt][:, n0:n0 + nsz],
                                 start=(kt == 0), stop=(kt == kd - 1))
            # silu(h1)*h3 = sigmoid(h1)*h3*h1 (CoreSim has no Silu LUT).
            # Both ACT ops read PSUM (wait on PE); both DVE ops then wait on
            # a single engine each — the DVE TensorTensor encoding only has
            # room for one sync-wait command.
            s = sp.tile([P, nsz], f32, tag="s", name=f"s_{ph}_{m}_{n0}")
            nc.scalar.activation(s[:], p1[:],
                                 mybir.ActivationFunctionType.Sigmoid)
            c3 = sp.tile([P, nsz], f32, tag="c3", name=f"c3_{ph}_{m}_{n0}")
            nc.scalar.copy(c3[:], p3[:])
            t = sp.tile([P, nsz], f32, tag="t", name=f"t_{ph}_{m}_{n0}")
            nc.vector.tensor_mul(t[:], s[:], c3[:])
            nc.vector.tensor_mul(hpm[:, n0:n0 + nsz], t[:], p1[:])
        hp.append(hpm)

    # ---- phase 2: out[tok] = comb * (h'.T @ W2) ----
    w2t = []
    for kt in range(n_itiles):
        w = w2p.tile([P, d_out], dt, tag=f"w2_{kt}", name=f"w2_{ph}_{kt}")
        nc.sync.dma_start(out=w[:], in_=w2_dram[kt])
        w2t.append(w)

    # Cycle PSUM tags so phase 2 rotates through all 8 banks (phase 1's
    # p1/p3 slots are idle here).
    ps2 = [(psY, "py"), (psY, "py"), (psY, "py"), (psY, "py"),
           (psA, "p1"), (psA, "p1"), (psA, "p3"), (psA, "p3")]
    idx = 0
    for mt in range((cp + P - 1) // P):
        msz = min(P, cp - mt * P)
        osb = op.tile([P, d_out], f32, tag="osb", name=f"osb_{ph}_{mt}")
        for n0 in range(0, d_out, NSZ):
            nn = min(NSZ, d_out - n0)
            pool, ptag = ps2[idx % len(ps2)]
            idx += 1
            py = pool.tile([P, nn], f32, tag=ptag, name=f"py_{ph}_{mt}_{n0}")
            for kt in range(n_itiles):
                nc.tensor.matmul(py[:msz], hp[kt][:, mt * P:mt * P + msz],
                                 w2t[kt][:, n0:n0 + nn],
                                 start=(kt == 0), stop=(kt == n_itiles - 1))
            if cgt is not None:
                nc.vector.tensor_scalar_mul(osb[:msz, n0:n0 + nn], py[:msz],
                                            cgt[:msz, mt:mt + 1])
            else:
                nc.vector.tensor_copy(osb[:msz, n0:n0 + nn], py[:msz])
        nc.sync.dma_start(out=out_rows[mt][:msz], in_=osb[:msz])


def build_program(C, cfg):
    """Build the per-core Bass program. C = routed token capacity (mult of 128)."""
    nc = bacc.Bacc()
    dt = mybir.dt.bfloat16
    f32 = mybir.dt.float32
    dv = _derived(cfg)
    epc, kd, it_r, it_s, nch = dv["epc"], dv["kd"], dv["it_r"], dv["it_s"], dv["nch"]
    tch = cfg["t_chunk"]
    d_out = cfg["d_out"]

    n_mt = (C + P - 1) // P
    ins = {}
    for j in range(epc):
        ins[f"xt{j}"] = nc.dram_tensor(f"xt{j}", [kd, P, C], dt, kind="ExternalInput")
        ins[f"w1_{j}"] = nc.dram_tensor(f"w1_{j}", [it_r, P, kd, P], dt, kind="ExternalInput")
        ins[f"w3_{j}"] = nc.dram_tensor(f"w3_{j}", [it_r, P, kd, P], dt, kind="ExternalInput")
        ins[f"w2_{j}"] = nc.dram_tensor(f"w2_{j}", [it_r, P, d_out], dt, kind="ExternalInput")
        ins[f"cg{j}"] = nc.dram_tensor(f"cg{j}", [P, n_mt], f32, kind="ExternalInput")
    for ch in range(nch):
        ins[f"xts{ch}"] = nc.dram_tensor(f"xts{ch}", [kd, P, tch], dt, kind="ExternalInput")
    ins["ws1"] = nc.dram_tensor("ws1", [it_s, P, kd, P], dt, kind="ExternalInput")
    ins["ws3"] = nc.dram_tensor("ws3", [it_s, P, kd, P], dt, kind="ExternalInput")
    ins["ws2"] = nc.dram_tensor("ws2", [it_s, P, d_out], dt, kind="ExternalInput")

    outs = {}
    for j in range(epc):
        outs[f"y{j}"] = nc.dram_tensor(f"y{j}", [n_mt, P, d_out], f32, kind="ExternalOutput")
    outs["z"] = nc.dram_tensor("z", [cfg["T"] // P, P, d_out], f32, kind="ExternalOutput")

    with tile.TileContext(nc) as tc:
        with (
            tc.tile_pool(name="xt", bufs=2) as xtp,
            tc.tile_pool(name="w", bufs=2) as wp,
            tc.tile_pool(name="hp", bufs=1) as hpp,
            tc.tile_pool(name="w2", bufs=1) as w2p,
            tc.tile_pool(name="s", bufs=3) as sp,
            tc.tile_pool(name="o", bufs=3) as op,
            tc.tile_pool(name="cg", bufs=2) as cgp,
            tc.tile_pool(name="psA", bufs=2, space="PSUM") as psA,
            tc.tile_pool(name="psY", bufs=4, space="PSUM") as psY,
        ):
            pools = dict(xt=xtp, w=wp, hp=hpp, w2=w2p, s=sp, o=op, cg=cgp,
                         psA=psA, psY=psY)
            for j in range(epc):
                _emit_phase(nc, pools, ins[f"xt{j}"], ins[f"w1_{j}"],
                            ins[f"w3_{j}"], ins[f"w2_{j}"], ins[f"cg{j}"],
                            [outs[f"y{j}"][mt] for mt in range(n_mt)],
                            it_r, C, cfg, ph=f"e{j}")
            for ch in range(nch):
                mts = tch // P
                _emit_phase(nc, pools, ins[f"xts{ch}"], ins["ws1"],
                            ins["ws3"], ins["ws2"], None,
                            [outs["z"][ch * mts + mt] for mt in range(mts)],
                            it_s, tch, cfg, ph=f"s{ch}")
    nc.compile()
    return nc


def _panelize_w13(w, itiles):
    """(D, I) -> (itiles, 128, kd, 128) so each M-panel is one contiguous DMA."""
    dd, ii = w.shape
    return np.ascontiguousarray(
        w.reshape(dd // P, P, itiles, P).transpose(2, 1, 0, 3))


def prep(x, weights, indices, W1, W3, W2, Ws1, Ws3, Ws2, cfg, force_C=None):
    """Host-side dispatch: shard/gather/pad/cast/pre-tile all inputs."""
    T, D, E = cfg["T"], cfg["D"], cfg["E"]
    dv = _derived(cfg)
    epc, kd, it_r, it_s, nch = dv["epc"], dv["kd"], dv["it_r"], dv["it_s"], dv["nch"]
    tch = cfg["t_chunk"]
    ish_c, ish_pad = cfg["ish_c"], cfg["ish_pad"]

    xf = np.asarray(x, F32).reshape(T, D)
    wts = np.asarray(weights, F32)
    idx = np.asarray(indices).astype(np.int64)
    W1 = np.asarray(W1, F32)
    W3 = np.asarray(W3, F32)
    W2 = np.asarray(W2, F32)
    Ws1 = np.asarray(Ws1, F32)
    Ws3 = np.asarray(Ws3, F32)
    Ws2 = np.asarray(Ws2, F32)

    # Per-(token, expert) combine weight; duplicate expert ids accumulate.
    comb = np.zeros((T, E), F32)
    np.add.at(comb, (np.arange(T)[:, None], idx), wts)

    # Token dispatch (host-side all-to-all): gather token ids per expert.
    tok_lists = [np.nonzero((idx == e).any(axis=1))[0] for e in range(E)]
    counts = [len(t) for t in tok_lists]
    C = max(NSZ, -(-max(counts) // 64) * 64) if force_C is None else force_C
    assert C >= max(counts)
    n_mt = (C + P - 1) // P

    xT16 = np.ascontiguousarray(xf.T).astype(BF16)  # (D, T)

    in_maps = []
    for c in range(cfg["ncores"]):
        m = {}
        for j in range(epc):
            e = epc * c + j
            toks = tok_lists[e]
            tpad = np.zeros(C, np.int64)
            tpad[:counts[e]] = toks
            m[f"xt{j}"] = np.ascontiguousarray(xT16[:, tpad].reshape(kd, P, C))
            m[f"w1_{j}"] = _panelize_w13(W1[e], it_r).astype(BF16)
            m[f"w3_{j}"] = _panelize_w13(W3[e], it_r).astype(BF16)
            m[f"w2_{j}"] = np.ascontiguousarray(W2[e].reshape(it_r, P, cfg["d_out"])).astype(BF16)
            cg = np.zeros(n_mt * P, F32)
            cg[:counts[e]] = comb[toks, e]
            m[f"cg{j}"] = np.ascontiguousarray(cg.reshape(n_mt, P).T)
        for ch in range(nch):
            m[f"xts{ch}"] = np.ascontiguousarray(
                xT16[:, ch * tch:(ch + 1) * tch].reshape(kd, P, tch))
        s0 = c * ish_c
        ws1 = np.zeros((D, ish_pad), F32)
        ws1[:, :ish_c] = Ws1[:, s0:s0 + ish_c]
        ws3 = np.zeros((D, ish_pad), F32)
        ws3[:, :ish_c] = Ws3[:, s0:s0 + ish_c]
        ws2 = np.zeros((ish_pad, cfg["d_out"]), F32)
        ws2[:ish_c] = Ws2[s0:s0 + ish_c]
        m["ws1"] = _panelize_w13(ws1, it_s).astype(BF16)
        m["ws3"] = _panelize_w13(ws3, it_s).astype(BF16)
        m["ws2"] = np.ascontiguousarray(ws2.reshape(it_s, P, cfg["d_out"])).astype(BF16)
        in_maps.append(m)

    meta = dict(tok_lists=tok_lists, counts=counts, C=C)
    return in_maps, meta


def combine(results, meta, cfg):
    """Host-side unshard: sum shared partials, scatter-add routed outputs."""
    T, D = cfg["T"], cfg["d_out"]
    epc = _derived(cfg)["epc"]
    out = np.zeros((T, D), F32)
    for c in range(cfg["ncores"]):
        r = results[c]
        out += r["z"].reshape(T, D)
        for j in range(epc):
            e = epc * c + j
            ye = r[f"y{j}"].reshape(-1, D)[:meta["counts"][e]]
            out[meta["tok_lists"][e]] += ye
    return out


# Test-harness knobs (kernel() callers get no-trace defaults).
TRACE = False
TMPDIR = None
LAST_RESULT = None


def kernel(x, weights, indices, W1, W3, W2, Ws1, Ws3, Ws2):
    global LAST_RESULT
    cfg = FULL_CFG
    in_maps, meta = prep(x, weights, indices, W1, W3, W2,
                         Ws1, Ws3, Ws2, cfg)
    nc = build_program(meta["C"], cfg)
    res = run_bass_kernel_spmd(nc, in_maps, core_ids=list(range(cfg["ncores"])),
                               trace=TRACE, tmpdir=TMPDIR)
    LAST_RESULT = res
    out = combine(res.results, meta, cfg)
    return out.reshape(1, cfg["T"], cfg["D"]).astype(F32)
